# revision 39
# baseline (speedup 1.0000x reference)
"""SkipGram negative-sampling loss kernel for 8 Trainium2 NeuronCores.

Strategy: data-parallel over walks (batch). The 1M x 128 embedding table is
replicated to every core's HBM; each core handles B/8 = 128 walks (one walk
per SBUF partition):
  - 6 large indirect-DMA gathers (walk split in two so the first pos chunk
    starts ~6us earlier; one per neg plane) with f32->bf16 cast during DMA.
    Large gathers amortize the ~1.1us SWDGE per-instruction overhead that
    dominated a per-column gather (384 instructions -> 6).
  - dot products per plane: bf16 tensor_tensor multiply (2x DVE mode) +
    halving-add tree down to width 2 (tensor_tensor 2x beats the 1x-mode
    tensor_reduce), final pair-add emits f32 logits.
  - stable softplus on the Scalar engine: relu/abs/exp for the pos block as
    soon as pos logits are done, neg block in the tail, one Ln pass over
    everything; per-partition sums via activation accum_out columns. The
    Abs/Exp/Relu table is preloaded at t=0 by a dummy activation.
  - each core returns [128, 1] partial sums; host sums and divides.

Engine notes (measured): DVE is the bottleneck (~87us busy); tensor_tensor
bf16 runs 2x (1.9 elem/ns/partition), tensor_reduce only 1x. GpSimd
tensor_add is ~2.5ns/elem AND steals SBUF ports from DVE - do not offload
tree levels there. Three indirect-DMA writers into one SBUF tile corrupt
data on HW (two are fine).
"""

import sys
import types

import numpy as np

try:  # missing in some containers; shim so trace=True degrades gracefully
    from antenv.axon_hooks import get_axon_ntff_profile_hook  # noqa: F401
except Exception:
    _m = types.ModuleType("antenv.axon_hooks")
    _m.get_axon_ntff_profile_hook = lambda: None
    sys.modules["antenv.axon_hooks"] = _m

import concourse.bass as bass
import concourse.bacc as bacc
import concourse.tile as tile
import concourse.mybir as mybir
from concourse.bass_utils import run_bass_kernel_spmd

F32 = mybir.dt.float32
BF16 = mybir.dt.bfloat16
I32 = mybir.dt.int32

N_CORES = 8


def build_kernel(n_walks, L, A, NEG, D, n_nodes, n_cores=N_CORES):
    """Build the SPMD Bass module (same NEFF on every core)."""
    W1 = L - A  # window_size - 1 = number of pos offsets (4)
    H = A  # anchors per chunk (full planes; halving costs more in overhead)
    NCH = W1 + NEG  # 8 chunks
    nc = bacc.Bacc(
        "TRN2",
        target_bir_lowering=False,
        debug=False,
        num_devices=n_cores,
    )
    walk_idx = nc.dram_tensor("walk_idx", [n_walks, L], I32, kind="ExternalInput")
    neg_idx = nc.dram_tensor("neg_idx", [n_walks, NEG * A], I32, kind="ExternalInput")
    embed = nc.dram_tensor("embed", [n_nodes, D], F32, kind="ExternalInput")
    out = nc.dram_tensor("out", [n_walks, 1], F32, kind="ExternalOutput")

    with tile.TileContext(nc) as tc:
        with (
            tc.tile_pool(name="idx", bufs=1) as idxp,
            tc.tile_pool(name="ew", bufs=1) as ewp,
            tc.tile_pool(name="en", bufs=4) as enp,
            tc.tile_pool(name="prod", bufs=3) as prodp,
            tc.tile_pool(name="t1", bufs=2) as t1p,
            tc.tile_pool(name="t2", bufs=2) as t2p,
            tc.tile_pool(name="sp", bufs=1) as spp,
            tc.tile_pool(name="small", bufs=1) as smallp,
        ):
            # Abs table warmup: tiny activation at t=0 so the first ACT table
            # load happens during the gather phase, not on the critical tail
            warm = smallp.tile([n_walks, 4], F32)
            warm2 = smallp.tile([n_walks, 4], F32)
            nc.gpsimd.memset(warm[:], 0.0)
            nc.scalar.activation(
                warm2[:], warm[:], mybir.ActivationFunctionType.Abs
            )

            wi = idxp.tile([n_walks, L], I32)
            nc.sync.dma_start(out=wi[:], in_=walk_idx[:])
            ni = idxp.tile([n_walks, NEG * A], I32)
            nc.sync.dma_start(out=ni[:], in_=neg_idx[:])

            # walk gather, split so the first pos half-plane can start early
            # (three-way splits break on HW: three indirect-DMA writers into
            # one tile produced garbage reads — keep exactly two)
            WS = 39
            ew16 = ewp.tile([n_walks, L * D], BF16)
            nc.gpsimd.indirect_dma_start(
                out=ew16[:, 0 : WS * D],
                out_offset=None,
                in_=embed[:],
                in_offset=bass.IndirectOffsetOnAxis(ap=wi[:, 0:WS], axis=0),
            )
            nc.gpsimd.indirect_dma_start(
                out=ew16[:, WS * D :],
                out_offset=None,
                in_=embed[:],
                in_offset=bass.IndirectOffsetOnAxis(ap=wi[:, WS:L], axis=0),
            )
            # one gather per neg plane
            en16 = []
            for j in range(NEG):
                t = enp.tile([n_walks, A * D], BF16)
                nc.gpsimd.indirect_dma_start(
                    out=t[:],
                    out_offset=None,
                    in_=embed[:],
                    in_offset=bass.IndirectOffsetOnAxis(
                        ap=ni[:, j * A : (j + 1) * A], axis=0
                    ),
                )
                en16.append(t)

            # chunk list: (anc_ap, other_ap, n_anchors). The first pos plane
            # is split at anchor 38 so chunk 0 only needs walk cols < WS.
            HA = 38
            chunks = [
                (ew16[:, 0 : HA * D], ew16[:, 1 * D : (1 + HA) * D], HA),
                (ew16[:, HA * D : A * D], ew16[:, (1 + HA) * D : (1 + A) * D], HA),
            ]
            for i in range(2, W1 + 1):
                chunks.append((ew16[:, 0 : A * D], ew16[:, i * D : (i + A) * D], A))
            for j in range(NEG):
                chunks.append((ew16[:, 0 : A * D], en16[j][:], A))
            NL = sum(c[2] for c in chunks)  # total logit columns
            PL = 2 * HA + (W1 - 1) * A  # pos block size
            offs = [0]
            for c in chunks:
                offs.append(offs[-1] + c[2])

            logits = smallp.tile([n_walks, NL], F32)

            def mult(k):
                anc, other, na = chunks[k]
                prod = prodp.tile([n_walks, A * D], BF16)
                nc.vector.tensor_mul(prod[:, 0 : na * D], anc, other)
                return prod

            def tail(k, prod):
                na = chunks[k][2]
                cur = prod[:, 0 : na * D].rearrange("p (a d) -> p a d", d=D)
                w = D
                while w > 2:
                    h2 = w // 2
                    pool = t1p if w == D else t2p
                    nt = pool.tile([n_walks, A * h2], BF16, tag=f"t{h2}")
                    n3 = nt[:, 0 : na * h2].rearrange("p (a d) -> p a d", d=h2)
                    nc.vector.tensor_add(n3, cur[:, :, 0:h2], cur[:, :, h2:w])
                    cur = n3
                    w = h2
                # final pair-add writes the f32 logits directly (cheaper at
                # 2x than a 1x-mode tensor_reduce over a wider tail)
                nc.vector.tensor_add(
                    logits[:, offs[k] : offs[k + 1]],
                    cur[:, :, 0],
                    cur[:, :, 1],
                )

            # tail(k) is emitted two multiplies later to keep DVE fed while
            # gather-gated multiplies wait
            prods = {}
            NCHUNK = len(chunks)
            LAG = 2
            for k in range(NCHUNK):
                prods[k] = mult(k)
                if k - LAG >= 0:
                    tail(k - LAG, prods.pop(k - LAG))
            for k in range(NCHUNK - LAG, NCHUNK):
                tail(k, prods.pop(k))

            # batched stable softplus over all logits, fully on ACT:
            #   softplus(s*x) = relu(s*x) + ln(1 + exp(-|x|))
            # (s = -1 for pos chunks, +1 for neg). The three partial sums
            # land in accum columns; a tiny DVE reduce finishes.
            # Two chains so the pos block (ready much earlier) runs off the
            # critical path; only the neg block remains in the tail. Order
            # keeps shared-table funcs (Relu/Abs/Exp) together per block so
            # only the two Ln switches load a table, the first one early.
            acc = smallp.tile([n_walks, 3], F32)
            scr = spp.tile([n_walks, NL], F32)
            ab = spp.tile([n_walks, NL], F32)
            e = spp.tile([n_walks, NL], F32)

            def sp_pre(c0, c1, sign, acol):
                """relu/abs/exp for one block — all share one ACT table"""
                sl = slice(c0, c1)
                nc.scalar.activation(
                    scr[:, sl], logits[:, sl],
                    mybir.ActivationFunctionType.Relu,
                    scale=sign, accum_out=acc[:, acol : acol + 1],
                )
                nc.scalar.activation(
                    ab[:, sl], logits[:, sl], mybir.ActivationFunctionType.Abs
                )
                nc.scalar.activation(
                    e[:, sl], ab[:, sl],
                    mybir.ActivationFunctionType.Exp, scale=-1.0,
                )

            sp_pre(0, PL, -1.0, 0)   # pos block: ready early, off the tail
            sp_pre(PL, NL, 1.0, 2)   # neg block: table already resident
            # one Ln over the whole row: depends on both exp halves (so the
            # scheduler can't hoist it into a table ping-pong), single load
            nc.scalar.activation(
                scr[:], e[:],
                mybir.ActivationFunctionType.Ln, bias=1.0,
                accum_out=acc[:, 1:2],
            )
            osum = smallp.tile([n_walks, 1], F32)
            nc.vector.tensor_reduce(
                osum[:],
                acc[:],
                axis=mybir.AxisListType.X,
                op=mybir.AluOpType.add,
            )
            nc.sync.dma_start(out=out[:], in_=osum[:])

    nc.compile()
    return nc


_NC_CACHE = {}


def _get_nc(key):
    if key not in _NC_CACHE:
        _NC_CACHE[key] = build_kernel(*key)
    return _NC_CACHE[key]


def make_in_maps(walk, neg, embed, n_cores=N_CORES):
    B, L = walk.shape
    A, NEG = neg.shape[1], neg.shape[2]
    nw = B // n_cores
    embed_f = np.ascontiguousarray(embed.astype(np.float32, copy=False))
    in_maps = []
    for c in range(n_cores):
        sl = slice(c * nw, (c + 1) * nw)
        wslice = np.ascontiguousarray(walk[sl].astype(np.int32, copy=False))
        # neg [nw, A, NEG] -> plane-major [nw, NEG*A]
        nslice = np.ascontiguousarray(
            neg[sl].astype(np.int32, copy=False).transpose(0, 2, 1).reshape(nw, NEG * A)
        )
        in_maps.append({"walk_idx": wslice, "neg_idx": nslice, "embed": embed_f})
    return in_maps


def kernel(walk, neg, embed, _trace=False):
    walk = np.asarray(walk)
    neg = np.asarray(neg)
    embed = np.asarray(embed)
    B, L = walk.shape
    A, NEG = neg.shape[1], neg.shape[2]
    n_nodes, D = embed.shape

    nc = _get_nc((B // N_CORES, L, A, NEG, D, n_nodes, N_CORES))
    in_maps = make_in_maps(walk, neg, embed)
    res = run_bass_kernel_spmd(
        nc, in_maps, core_ids=list(range(N_CORES)), trace=_trace
    )
    total = 2 * B * A * NEG
    s = sum(r["out"].astype(np.float64).sum() for r in res.results)
    loss = np.float32(s / total)
    if _trace:
        return loss, res
    return loss


# revision 43
# speedup vs baseline: 1.0340x; 1.0340x over previous
"""SkipGram negative-sampling loss kernel for 8 Trainium2 NeuronCores.

Strategy: data-parallel over walks (batch). The 1M x 128 embedding table is
replicated to every core's HBM; each core handles B/8 = 128 walks (one walk
per SBUF partition):
  - 6 large indirect-DMA gathers (walk split in two so the first pos chunk
    starts ~6us earlier; one per neg plane) with f32->bf16 cast during DMA.
    Large gathers amortize the ~1.1us SWDGE per-instruction overhead that
    dominated a per-column gather (384 instructions -> 6).
  - dot products per plane: bf16 tensor_tensor multiply (2x DVE mode) +
    halving-add tree down to width 2 (tensor_tensor 2x beats the 1x-mode
    tensor_reduce), final pair-add emits f32 logits.
  - stable softplus on the Scalar engine: relu/abs/exp for the pos block as
    soon as pos logits are done, neg block in the tail, one Ln pass over
    everything; per-partition sums via activation accum_out columns. The
    Abs/Exp/Relu table is preloaded at t=0 by a dummy activation.
  - each core returns [128, 1] partial sums; host sums and divides.

Engine notes (measured): DVE is the bottleneck (~87us busy); tensor_tensor
bf16 runs 2x (1.9 elem/ns/partition), tensor_reduce only 1x. GpSimd
tensor_add is ~2.5ns/elem AND steals SBUF ports from DVE - do not offload
tree levels there. Three indirect-DMA writers into one SBUF tile corrupt
data on HW (two are fine).
"""

import sys
import types

import numpy as np

try:  # missing in some containers; shim so trace=True degrades gracefully
    from antenv.axon_hooks import get_axon_ntff_profile_hook  # noqa: F401
except Exception:
    _m = types.ModuleType("antenv.axon_hooks")
    _m.get_axon_ntff_profile_hook = lambda: None
    sys.modules["antenv.axon_hooks"] = _m

import concourse.bass as bass
import concourse.bacc as bacc
import concourse.tile as tile
import concourse.mybir as mybir
from concourse.bass_utils import run_bass_kernel_spmd

F32 = mybir.dt.float32
BF16 = mybir.dt.bfloat16
I32 = mybir.dt.int32

N_CORES = 8


def build_kernel(n_walks, L, A, NEG, D, n_nodes, n_cores=N_CORES):
    """Build the SPMD Bass module (same NEFF on every core)."""
    W1 = L - A  # window_size - 1 = number of pos offsets (4)
    H = A  # anchors per chunk (full planes; halving costs more in overhead)
    NCH = W1 + NEG  # 8 chunks
    nc = bacc.Bacc(
        "TRN2",
        target_bir_lowering=False,
        debug=False,
        num_devices=n_cores,
    )
    walk_idx = nc.dram_tensor("walk_idx", [n_walks, L], I32, kind="ExternalInput")
    neg_idx = nc.dram_tensor("neg_idx", [n_walks, NEG * A], I32, kind="ExternalInput")
    embed = nc.dram_tensor("embed", [n_nodes, D], F32, kind="ExternalInput")
    out = nc.dram_tensor("out", [n_walks, 1], F32, kind="ExternalOutput")

    with tile.TileContext(nc) as tc:
        with (
            tc.tile_pool(name="idx", bufs=1) as idxp,
            tc.tile_pool(name="ew", bufs=1) as ewp,
            tc.tile_pool(name="en", bufs=4) as enp,
            tc.tile_pool(name="prod", bufs=3) as prodp,
            tc.tile_pool(name="t1", bufs=2) as t1p,
            tc.tile_pool(name="t2", bufs=2) as t2p,
            tc.tile_pool(name="sp", bufs=1) as spp,
            tc.tile_pool(name="small", bufs=1) as smallp,
        ):
            # Abs table warmup: tiny activation at t=0 so the first ACT table
            # load happens during the gather phase, not on the critical tail
            warm = smallp.tile([n_walks, 4], F32)
            warm2 = smallp.tile([n_walks, 4], F32)
            nc.gpsimd.memset(warm[:], 0.0)
            nc.scalar.activation(
                warm2[:], warm[:], mybir.ActivationFunctionType.Abs
            )

            wi = idxp.tile([n_walks, L], I32)
            nc.sync.dma_start(out=wi[:], in_=walk_idx[:])
            ni = idxp.tile([n_walks, NEG * A], I32)
            nc.sync.dma_start(out=ni[:], in_=neg_idx[:])

            # walk gather, split so the first pos half-plane can start early
            # (three-way splits break on HW: three indirect-DMA writers into
            # one tile produced garbage reads — keep exactly two)
            WS = 39
            ew16 = ewp.tile([n_walks, L * D], BF16)
            nc.gpsimd.indirect_dma_start(
                out=ew16[:, 0 : WS * D],
                out_offset=None,
                in_=embed[:],
                in_offset=bass.IndirectOffsetOnAxis(ap=wi[:, 0:WS], axis=0),
            )
            nc.gpsimd.indirect_dma_start(
                out=ew16[:, WS * D :],
                out_offset=None,
                in_=embed[:],
                in_offset=bass.IndirectOffsetOnAxis(ap=wi[:, WS:L], axis=0),
            )
            # one gather per neg plane
            en16 = []
            for j in range(NEG):
                t = enp.tile([n_walks, A * D], BF16)
                nc.gpsimd.indirect_dma_start(
                    out=t[:],
                    out_offset=None,
                    in_=embed[:],
                    in_offset=bass.IndirectOffsetOnAxis(
                        ap=ni[:, j * A : (j + 1) * A], axis=0
                    ),
                )
                en16.append(t)

            # chunk list: (anc_ap, other_ap, n_anchors). Every pos plane is
            # split at the anchor where it stops fitting in walk part 1
            # (anchor a of plane i reads cols a and a+i, so a <= WS-1-i),
            # giving DVE ~20us of work before walk part 2 lands.
            chunks = []
            for i in range(1, W1 + 1):
                ne = WS - i  # early anchors [0, ne)
                chunks.append(
                    (ew16[:, 0 : ne * D], ew16[:, i * D : WS * D], ne)
                )
            for i in range(1, W1 + 1):
                ne = WS - i
                chunks.append(
                    (
                        ew16[:, ne * D : A * D],
                        ew16[:, WS * D : (A + i) * D],
                        A - ne,
                    )
                )
            for j in range(NEG):
                chunks.append((ew16[:, 0 : A * D], en16[j][:], A))
            NL = sum(c[2] for c in chunks)  # total logit columns
            PL = W1 * A  # pos block size (pos chunk widths sum to W1*A)
            offs = [0]
            for c in chunks:
                offs.append(offs[-1] + c[2])

            logits = smallp.tile([n_walks, NL], F32)

            def mult(k):
                anc, other, na = chunks[k]
                prod = prodp.tile([n_walks, A * D], BF16)
                nc.vector.tensor_mul(prod[:, 0 : na * D], anc, other)
                return prod

            def tail(k, prod):
                na = chunks[k][2]
                cur = prod[:, 0 : na * D].rearrange("p (a d) -> p a d", d=D)
                w = D
                while w > 2:
                    h2 = w // 2
                    pool = t1p if w == D else t2p
                    nt = pool.tile([n_walks, A * h2], BF16, tag=f"t{h2}")
                    n3 = nt[:, 0 : na * h2].rearrange("p (a d) -> p a d", d=h2)
                    nc.vector.tensor_add(n3, cur[:, :, 0:h2], cur[:, :, h2:w])
                    cur = n3
                    w = h2
                # final pair-add writes the f32 logits directly (cheaper at
                # 2x than a 1x-mode tensor_reduce over a wider tail)
                nc.vector.tensor_add(
                    logits[:, offs[k] : offs[k + 1]],
                    cur[:, :, 0],
                    cur[:, :, 1],
                )

            # tail(k) is emitted two multiplies later to keep DVE fed while
            # gather-gated multiplies wait
            prods = {}
            NCHUNK = len(chunks)
            LAG = 2
            for k in range(NCHUNK):
                prods[k] = mult(k)
                if k - LAG >= 0:
                    tail(k - LAG, prods.pop(k - LAG))
            for k in range(NCHUNK - LAG, NCHUNK):
                tail(k, prods.pop(k))

            # batched stable softplus over all logits, fully on ACT:
            #   softplus(s*x) = relu(s*x) + ln(1 + exp(-|x|))
            # (s = -1 for pos chunks, +1 for neg). The three partial sums
            # land in accum columns; a tiny DVE reduce finishes.
            # Two chains so the pos block (ready much earlier) runs off the
            # critical path; only the neg block remains in the tail. Order
            # keeps shared-table funcs (Relu/Abs/Exp) together per block so
            # only the two Ln switches load a table, the first one early.
            acc = smallp.tile([n_walks, 4], F32)
            scr = spp.tile([n_walks, NL], F32)
            ab = spp.tile([n_walks, NL], F32)
            e = spp.tile([n_walks, NL], F32)

            def sp_pre(c0, c1, sign, acol):
                """relu/abs/exp for one block — all share one ACT table"""
                sl = slice(c0, c1)
                nc.scalar.activation(
                    scr[:, sl], logits[:, sl],
                    mybir.ActivationFunctionType.Relu,
                    scale=sign, accum_out=acc[:, acol : acol + 1],
                )
                nc.scalar.activation(
                    ab[:, sl], logits[:, sl], mybir.ActivationFunctionType.Abs
                )
                nc.scalar.activation(
                    e[:, sl], ab[:, sl],
                    mybir.ActivationFunctionType.Exp, scale=-1.0,
                )

            sp_pre(0, PL, -1.0, 0)   # pos block: ready early, off the tail
            # neg block split: n0-n2 run while DVE finishes n3's tree, so
            # only n3's 76 columns remain in the serial tail
            NB = PL + (NEG - 1) * A
            sp_pre(PL, NB, 1.0, 2)
            sp_pre(NB, NL, 1.0, 3)
            # one Ln over the whole row: depends on both exp halves (so the
            # scheduler can't hoist it into a table ping-pong), single load
            nc.scalar.activation(
                scr[:], e[:],
                mybir.ActivationFunctionType.Ln, bias=1.0,
                accum_out=acc[:, 1:2],
            )
            osum = smallp.tile([n_walks, 1], F32)
            nc.vector.tensor_reduce(
                osum[:],
                acc[:],
                axis=mybir.AxisListType.X,
                op=mybir.AluOpType.add,
            )
            nc.sync.dma_start(out=out[:], in_=osum[:])

    nc.compile()
    return nc


_NC_CACHE = {}


def _get_nc(key):
    if key not in _NC_CACHE:
        _NC_CACHE[key] = build_kernel(*key)
    return _NC_CACHE[key]


def make_in_maps(walk, neg, embed, n_cores=N_CORES):
    B, L = walk.shape
    A, NEG = neg.shape[1], neg.shape[2]
    nw = B // n_cores
    embed_f = np.ascontiguousarray(embed.astype(np.float32, copy=False))
    in_maps = []
    for c in range(n_cores):
        sl = slice(c * nw, (c + 1) * nw)
        wslice = np.ascontiguousarray(walk[sl].astype(np.int32, copy=False))
        # neg [nw, A, NEG] -> plane-major [nw, NEG*A]
        nslice = np.ascontiguousarray(
            neg[sl].astype(np.int32, copy=False).transpose(0, 2, 1).reshape(nw, NEG * A)
        )
        in_maps.append({"walk_idx": wslice, "neg_idx": nslice, "embed": embed_f})
    return in_maps


def kernel(walk, neg, embed, _trace=False):
    walk = np.asarray(walk)
    neg = np.asarray(neg)
    embed = np.asarray(embed)
    B, L = walk.shape
    A, NEG = neg.shape[1], neg.shape[2]
    n_nodes, D = embed.shape

    nc = _get_nc((B // N_CORES, L, A, NEG, D, n_nodes, N_CORES))
    in_maps = make_in_maps(walk, neg, embed)
    res = run_bass_kernel_spmd(
        nc, in_maps, core_ids=list(range(N_CORES)), trace=_trace
    )
    total = 2 * B * A * NEG
    s = sum(r["out"].astype(np.float64).sum() for r in res.results)
    loss = np.float32(s / total)
    if _trace:
        return loss, res
    return loss
